# revision 48
# baseline (speedup 1.0000x reference)
"""Fused LayerNorm + 16-head attention (b=2, n=2048, d=1024) on 8 trn2 cores.

Sharding: core c handles batch c//4 and heads [4*(c%4), 4*(c%4)+4).
Each core computes LayerNorm + QKV projection for its batch, attention for
its 4 heads, and a partial w_out product; the host sums the 4 partials per
batch.

Pipeline (all "transposed" layouts, contraction always on the partition dim):
  - host sends x^T [1024, 2048] fp16 and mask^T [2048, 2048] fp16 per batch
  - LN stats (mean/var over dim) via ones-matmuls on the PE
  - LN scale and gamma/beta are folded into the QKV projection:
      qkv^T[c, r] = sum_d W'[d,c] * (x^T[d,r]*rstd[r]) + wsum[c]*(-mu*rstd)[r]
                    + bsum[c]*1
    (W' = gamma*w_qkv columns for this core; wsum/bsum host-precomputed)
  - dots^T[j, i] per head via fp16 matmul (K=64)
  - e1 = exp(SCALE*dots) (ACT, psum->sbuf fp16); e2 = e1*mask^T (DVE fp16 2x)
  - U2[d|1, i] = sum_j [v|1][j,d] * e2[j,i]  (fp16 matmul, K=128 chunks),
    software-pipelined one j-chunk behind the dots matmuls
  - masked-fill-with-global-min handled exactly via
      U3 = U2 + em*(colsum_v - mask^T@v),  S3 = S2 + em*n_masked
    where em = exp(SCALE*min(dots)) = global min of e1 (running TT-min trees
    split across DVE and GPSIMD) + an 8-core AllReduce(min) collective.
  - out^T = U3 * (1/S3) ; partial = out^T.T @ w_out rows  (fp16 matmul)
"""

import numpy as np
import ml_dtypes

import concourse.bass as bass
import concourse.mybir as mybir
import concourse.tile as tile
from concourse import bacc
from concourse import bass_isa
from concourse.bass_utils import run_bass_kernel_spmd
from concourse.masks import make_identity

DIM, HEADS, DIM_HEAD = 1024, 16, 64
B, N = 2, 2048
INNER = HEADS * DIM_HEAD
SCALE = DIM_HEAD ** -0.5
EPS = 1e-5
NCORES = 8
GROUPS = NCORES // B          # head-groups per batch = 4
HPC = HEADS // GROUPS         # heads per core = 4
WCOLS = 3 * HPC * DIM_HEAD    # 768 qkv columns per core
KCH = DIM // 128              # 8 contraction chunks
NJC = N // 128                # 16 j-chunks
NIC = N // 512                # 4 i-chunks of 512
IH = 1024                     # i-half width

f32 = mybir.dt.float32
f16 = mybir.dt.float16
AX = mybir.AxisListType
ALU = mybir.AluOpType
ACTF = mybir.ActivationFunctionType

DEBUG = False


def build_nc():
    nc = bacc.Bacc(None, target_bir_lowering=False)

    xT = nc.dram_tensor("xT", [DIM, N], f16, kind="ExternalInput")
    maskT = nc.dram_tensor("maskT", [N, N], f16, kind="ExternalInput")
    nmask = nc.dram_tensor("nmask", [1, N], f32, kind="ExternalInput")
    wqkv = nc.dram_tensor("wqkv", [DIM, WCOLS], f16, kind="ExternalInput")
    wr1 = nc.dram_tensor("wr1", [2, WCOLS], f16, kind="ExternalInput")
    wout = nc.dram_tensor("wout", [2, 128, DIM], f16, kind="ExternalInput")
    out = nc.dram_tensor("out_part", [N, DIM], f32, kind="ExternalOutput")
    cc_in = nc.dram_tensor("cc_in", [1, 8], f32)
    cc_out = nc.dram_tensor("cc_out", [8, 8], f32, addr_space="Shared")
    scr_a = nc.dram_tensor("scr_a", [1, N], f32)
    scr_b = nc.dram_tensor("scr_b", [1, N], f16)
    scr_c = nc.dram_tensor("scr_c", [1, HPC * N], f32)
    scr_d = nc.dram_tensor("scr_d", [1, HPC * N], f16)
    scr_e = nc.dram_tensor("scr_e", [1, HPC * N], f32)
    scr_f = nc.dram_tensor("scr_f", [1, HPC * N], f16)
    dbg = {}
    if DEBUG:
        dbg["em"] = nc.dram_tensor("dbg_em", [1, 8], f32, kind="ExternalOutput")
        dbg["qk"] = nc.dram_tensor("dbg_qk", [128, 4, N], f16, kind="ExternalOutput")
        dbg["u2"] = nc.dram_tensor("dbg_u2", [64, HPC, 2, IH], f16, kind="ExternalOutput")
        dbg["s2"] = nc.dram_tensor("dbg_s2", [HPC, N], f32, kind="ExternalOutput")
        dbg["mv"] = nc.dram_tensor("dbg_mv", [128, 2, N], f16, kind="ExternalOutput")
        dbg["acc"] = nc.dram_tensor("dbg_acc", [128, IH], f16, kind="ExternalOutput")
        dbg["outT"] = nc.dram_tensor("dbg_outT", [128, 2, N], f16, kind="ExternalOutput")
        dbg["stats"] = nc.dram_tensor("dbg_stats", [2, N], f32, kind="ExternalOutput")
        dbg["rstd16"] = nc.dram_tensor("dbg_rstd16", [1, N], f16, kind="ExternalOutput")

    scr = (scr_a, scr_b, scr_c, scr_d, scr_e, scr_f)
    with tile.TileContext(nc) as tc:
        _build_body(nc, tc, xT, maskT, nmask, wqkv, wr1, wout, out,
                    cc_in, cc_out, scr, dbg)
    nc.compile()
    return nc


def _build_body(nc, tc, xT, maskT, nmask, wqkv, wr1, wout, out, cc_in, cc_out,
                scr, dbg):
    scr_a, scr_b, scr_c, scr_d, scr_e, scr_f = scr
    xT_t = xT.ap().rearrange("(ko p) r -> p ko r", p=128)          # [128, 8, N]
    maskT_t = maskT.ap().rearrange("(jo p) i -> p jo i", p=128)    # [128, 16, N]
    wqkv_t = wqkv.ap().rearrange("(ko p) c -> p ko c", p=128)      # [128, 8, 768]
    wout_t = wout.ap().rearrange("pr p od -> p pr od")             # [128, 2, 1024]
    out_t = out.ap().rearrange("(ro p) od -> p ro od", p=128)      # [128, 16, 1024]

    with tc.tile_pool(name="L1", bufs=1) as L1:
        # ---- whole-kernel tensors ----
        u2_sb = L1.tile([64, HPC, N], f16)             # U2 rows fp16 (16KB/p)
        s2st = L1.tile([65, IH], f32)                  # S-row staging (4KB/p)
        s2_sb = L1.tile([HPC, N], f32)
        mv_sb = L1.tile([128, 2, N], f16)
        outT_sb = L1.tile([128, 2, N], f16)
        identones = L1.tile([128, 132], f16)           # ident | ones col
        sm32 = L1.tile([128, 16], f32)   # 0:em_col 1:nem_col 2-5:cs 6-9:csm
                                         # [0,10]:eps [0,11]:local_min
        smrow = L1.tile([1, 16], f32)    # 0-7: cc_stage, 8-15: em_row

        ident = identones[:, 0:128]
        ones16 = identones[:, 128:129]
        em_col = sm32[:, 0:1]
        nem_col = sm32[:, 1:2]
        cs_sb = sm32[0:64, 2:6]
        csm = sm32[0:64, 6:10]
        eps_t = sm32[0:1, 10:11]
        local_min = sm32[0:1, 11:12]
        cc_stage = smrow[0:1, 0:8]
        em_row = smrow[0:1, 8:16]

        nc.vector.memset(identones, 0.0)
        make_identity(nc, ident, nomemset=True)
        nc.vector.memset(ones16, 1.0)
        nc.vector.memset(eps_t, EPS)

        with tc.tile_pool(name="L2", bufs=1) as L2:
            qk_sb = L2.tile([128, 4, N], f16)
            v_aug = L2.tile([128, NJC, 2, 2, 65], f16)
            v_pair = L2.tile([128, NJC, 256], f16)
            acc_min = L2.tile([128, IH], f16)
            w_sb = L2.tile([128, KCH, WCOLS], f16)
            wr1_sb = L2.tile([2, WCOLS], f16)
            nc.sync.dma_start(out=w_sb, in_=wqkv_t)
            nc.sync.dma_start(out=wr1_sb, in_=wr1.ap())
            nc.vector.memset(v_aug[:, :, :, :, 64:65], 1.0)
            nc.vector.memset(acc_min, 60000.0)
            lnout_cm = tc.tile_pool(name="LNOUT", bufs=1)
            LNOUT = lnout_cm.__enter__()
            rstd16 = LNOUT.tile([1, N], f16)
            r1 = LNOUT.tile([2, N], f16)
            x16 = LNOUT.tile([128, KCH, N], f16)
            # ================= Phase 1: LN stats =================
            with (
                tc.tile_pool(name="ph1ps", bufs=1, space="PSUM") as ph1ps,
                tc.tile_pool(name="ph1sb", bufs=1) as ph1sb,
                tc.tile_pool(name="xstream", bufs=3) as xstream,
            ):
                ps_sx = ph1ps.tile([1, N], f32)
                ps_sxx = ph1ps.tile([1, N], f32)
                for k in range(KCH):
                    nc.sync.dma_start(out=x16[:, k], in_=xT_t[:, k])
                    sq = xstream.tile([128, N], f16, tag="sq")
                    nc.vector.tensor_mul(out=sq, in0=x16[:, k], in1=x16[:, k])
                    for s in range(NIC):
                        sl = slice(s * 512, (s + 1) * 512)
                        nc.tensor.matmul(ps_sx[:, sl], ones16, x16[:, k, sl],
                                         start=(k == 0), stop=(k == KCH - 1))
                    for s in range(NIC):
                        sl = slice(s * 512, (s + 1) * 512)
                        nc.tensor.matmul(ps_sxx[:, sl], ones16, sq[:, sl],
                                         start=(k == 0), stop=(k == KCH - 1))
                # stats on partition 0
                mu = ph1sb.tile([1, N], f32)
                ex2 = ph1sb.tile([1, N], f32)
                mu2 = ph1sb.tile([1, N], f32)
                nc.vector.tensor_scalar_mul(out=mu, in0=ps_sx, scalar1=1.0 / DIM)
                nc.vector.tensor_scalar_mul(out=ex2, in0=ps_sxx, scalar1=1.0 / DIM)
                nc.scalar.activation(out=mu2, in_=mu, func=ACTF.Square)
                nc.vector.tensor_sub(out=ex2, in0=ex2, in1=mu2)     # var
                nc.scalar.activation(out=ex2, in_=ex2, func=ACTF.Sqrt,
                                     bias=eps_t)                    # sqrt(var+eps)
                # fast reciprocal: bounce through DRAM to use 128 partitions
                nc.sync.dma_start(out=scr_a.ap(), in_=ex2)
                rr = ph1sb.tile([128, N // 128], f32)
                nc.sync.dma_start(
                    out=rr, in_=scr_a.ap().rearrange("one (po f) -> (one po) f", po=128))
                nc.vector.reciprocal(out=rr, in_=rr)
                rr16 = ph1sb.tile([128, N // 128], f16)
                nc.vector.tensor_copy(out=rr16, in_=rr)
                nc.sync.dma_start(
                    out=scr_b.ap().rearrange("one (po f) -> (one po) f", po=128), in_=rr16)
                nc.sync.dma_start(out=rstd16, in_=scr_b.ap())
                # r1 row0 = -mu (f16); row1 = sd = 1/rstd (so that
                # rstd*(bsum*sd) = bsum at evac time)
                nc.vector.tensor_scalar_mul(out=mu2, in0=mu, scalar1=-1.0)
                nc.vector.tensor_copy(out=r1[0:1, :], in_=mu2)
                nc.gpsimd.dma_start(out=r1[1:2, :], in_=ex2)
                if DEBUG:
                    nc.sync.dma_start(out=dbg["stats"].ap()[0:1], in_=mu)
                    nc.sync.dma_start(out=dbg["rstd16"].ap(), in_=rstd16)

            # ================= Phase 2 wrapper =================
            if True:
                # ================= Phase 2: xhat + QKV projection ==============
                with tc.tile_pool(name="ph2sb", bufs=1) as ph2sb:
                    vT_sb = ph2sb.tile([128, 2, N], f16)
                    rstd_bc = ph2sb.tile([128, N], f16)
                    nc.gpsimd.partition_broadcast(rstd_bc, rstd16)

                    # proj on x16 directly; LN scale applied at psum evac.
                    # r1 row0 here is -mu (rstd folded at evac): see below.
                    ph2ps_cm = tc.tile_pool(name="ph2ps", bufs=2, space="PSUM")
                    ph2ps = ph2ps_cm.__enter__()
                    for cc in range(6):
                        ps_p = ph2ps.tile([128, N], f32, tag="proj")
                        ccs = slice(cc * 128, (cc + 1) * 128)
                        for k in range(KCH + 1):
                            lhsT = w_sb[:, k, ccs] if k < KCH else wr1_sb[:, ccs]
                            for s in range(NIC):
                                sl = slice(s * 512, (s + 1) * 512)
                                rhs = x16[:, k, sl] if k < KCH else r1[:, sl]
                                nc.tensor.matmul(ps_p[:, sl], lhsT, rhs,
                                                 start=(k == 0), stop=(k == KCH))
                        tgt = qk_sb[:, cc] if cc < 4 else vT_sb[:, cc - 4]
                        nc.vector.tensor_mul(out=tgt, in0=ps_p, in1=rstd_bc)

                    ph2ps_cm.__exit__(None, None, None)
                    # ---- v transposes -> v_aug natural layout
                    with tc.tile_pool(name="tps", bufs=2, space="PSUM") as tps:
                        for jc in range(NJC):
                            ps_t = tps.tile([128, 256], f16, tag="vt")
                            jsl = slice(jc * 128, (jc + 1) * 128)
                            for hc in range(2):
                                nc.tensor.transpose(
                                    ps_t[:, hc * 128:(hc + 1) * 128],
                                    vT_sb[:, hc, jsl], ident)
                            nc.vector.tensor_copy(
                                out=v_aug[:, jc, :, :, 0:64],
                                in_=ps_t.rearrange("p (pr m d) -> p pr m d",
                                                   pr=2, m=2))
                            nc.vector.tensor_copy(out=v_pair[:, jc], in_=ps_t)
                        # per-head column sums of v
                        for h in range(HPC):
                            ps_cs = tps.tile([64, 1], f32, tag="cs")
                            for jc in range(NJC):
                                nc.tensor.matmul(ps_cs,
                                                 v_aug[:, jc, h // 2, h % 2, 0:64],
                                                 ones16, start=(jc == 0),
                                                 stop=(jc == NJC - 1))
                            nc.vector.tensor_copy(out=cs_sb[:, h:h + 1], in_=ps_cs)

                lnout_cm.__exit__(None, None, None)
                # ================= Phase 3: attention (full mask resident) =====
                with (
                    tc.tile_pool(name="epool", bufs=4) as epool,
                    tc.tile_pool(name="maskp", bufs=1) as maskp,
                ):
                    mask_sb = maskp.tile([128, NJC, N], f16)
                    for jo in range(NJC):
                        nc.sync.dma_start(out=mask_sb[:, jo], in_=maskT_t[:, jo])
                    for ih in range(2):
                        isl = slice(ih * IH, (ih + 1) * IH)
                        psctx = (
                            (tc.tile_pool(name="dps0", bufs=2, space="PSUM"),
                             tc.tile_pool(name="ups0", bufs=1, space="PSUM"),
                             tc.tile_pool(name="mvps0", bufs=1, space="PSUM"))
                            if ih == 0 else
                            (tc.tile_pool(name="dps1", bufs=3, space="PSUM"),
                             tc.tile_pool(name="ups1", bufs=1, space="PSUM")))
                        pools = [cm.__enter__() for cm in psctx]
                        dps, ups = pools[0], pools[1]
                        mvps = pools[2] if ih == 0 else pools[0]
                        for h in range(HPC):
                            hc, off = h // 2, (h % 2) * 64
                            qT = qk_sb[off:off + 64, hc]       # [64, N]
                            kT = qk_sb[off:off + 64, 2 + hc]   # [64, N]
                            ps_u = ups.tile([128, IH], f32, tag="u")
                            mv_h = (ih == 0 and (h == 1 or h == 3))
                            if mv_h:
                                pr = h // 2
                                ps_mv = mvps.tile([128, IH], f32, tag="mv")
                            pend = []
                            for jc in range(NJC):
                                jsl = slice(jc * 128, (jc + 1) * 128)
                                ps_d = dps.tile([128, IH], f32, tag="d")
                                for s in range(2):
                                    ssl = slice(ih * IH + s * 512,
                                                ih * IH + (s + 1) * 512)
                                    nc.tensor.matmul(
                                        ps_d[:, s * 512:(s + 1) * 512],
                                        kT[:, jsl], qT[:, ssl],
                                        start=True, stop=True)
                                if mv_h:
                                    # fill PE bubble with this i-half's Mv
                                    for s in range(2):
                                        nc.tensor.matmul(
                                            ps_mv[:, s * 512:(s + 1) * 512],
                                            v_pair[:, jc,
                                                   pr * 128:(pr + 1) * 128],
                                            mask_sb[:, jc, ih * IH + s * 512:
                                                    ih * IH + (s + 1) * 512],
                                            start=(jc == 0),
                                            stop=(jc == NJC - 1))
                                # av lags by 2 j-chunks for PE density
                                if len(pend) == 2:
                                    pjc, pe2 = pend.pop(0)
                                    for s in range(2):
                                        nc.tensor.matmul(
                                            ps_u[0:65, s * 512:(s + 1) * 512],
                                            v_aug[:, pjc, hc, h % 2, :],
                                            pe2[:, s * 512:(s + 1) * 512],
                                            start=(pjc == 0), stop=False)
                                e1 = epool.tile([128, IH], f16, tag="e1")
                                nc.scalar.activation(out=e1, in_=ps_d,
                                                     func=ACTF.Exp, scale=SCALE)
                                e2 = epool.tile([128, IH], f16, tag="e2")
                                nc.vector.tensor_mul(out=e2, in0=e1,
                                                     in1=mask_sb[:, jc, isl])
                                nc.vector.tensor_tensor(
                                    out=acc_min, in0=acc_min,
                                    in1=e1, op=ALU.min)
                                pend.append((jc, e2))
                            for pjc, pe2 in pend:
                                for s in range(2):
                                    nc.tensor.matmul(
                                        ps_u[0:65, s * 512:(s + 1) * 512],
                                        v_aug[:, pjc, hc, h % 2, :],
                                        pe2[:, s * 512:(s + 1) * 512],
                                        start=(pjc == 0),
                                        stop=(pjc == NJC - 1))
                            nc.scalar.copy(out=u2_sb[:, h, isl],
                                           in_=ps_u[0:64])
                            s2t = epool.tile([65, IH], f32, tag="s2st",
                                             bufs=2)
                            nc.scalar.copy(out=s2t[64:65], in_=ps_u[64:65])
                            nc.sync.dma_start(out=s2_sb[h:h + 1, isl],
                                              in_=s2t[64:65])
                            if mv_h:
                                nc.scalar.copy(out=mv_sb[:, pr, isl],
                                               in_=ps_mv)
                        if ih == 1:
                            # trailing Mv chains: run while the global-min
                            # collective is in flight (no e1 dependency)
                            for pr in range(2):
                                ps_mv = dps.tile([128, IH], f32, tag="d")
                                for jc in range(NJC):
                                    for s in range(2):
                                        nc.tensor.matmul(
                                            ps_mv[:, s * 512:(s + 1) * 512],
                                            v_pair[:, jc,
                                                   pr * 128:(pr + 1) * 128],
                                            mask_sb[:, jc, ih * IH + s * 512:
                                                    ih * IH + (s + 1) * 512],
                                            start=(jc == 0),
                                            stop=(jc == NJC - 1))
                                nc.scalar.copy(out=mv_sb[:, pr, isl],
                                               in_=ps_mv)
                        for cm in reversed(psctx):
                            cm.__exit__(None, None, None)

                if DEBUG:
                    nc.sync.dma_start(out=dbg["qk"].ap(), in_=qk_sb)
                    nc.sync.dma_start(out=dbg["acc"].ap(), in_=acc_min)
                # collapse acc_min tree -> local_min scalar
                pm = L2.tile([128, 1], f32)
                nc.vector.tensor_reduce(out=pm, in_=acc_min, axis=AX.XYZW,
                                        op=ALU.min)
                nc.vector.tensor_scalar_mul(out=pm, in0=pm, scalar1=-1.0)
                pmx = L2.tile([128, 1], f32)
                nc.gpsimd.partition_all_reduce(pmx, pm, channels=128,
                                               reduce_op=bass_isa.ReduceOp.max)
                nc.vector.tensor_scalar_mul(out=local_min, in0=pmx[0:1],
                                            scalar1=-1.0)

        # ================= Phase 4: global-min collective =================
        mv_odd = L1.tile([64, 2, N], f16)
        for pr in range(2):
            nc.sync.dma_start(out=mv_odd[:, pr], in_=mv_sb[64:128, pr])
        nc.vector.memset(cc_stage, 3.0e38)
        nc.vector.tensor_copy(out=cc_stage[0:1, 0:1], in_=local_min)
        with tc.tile_critical(), nc.semaphore("ccd_sem") as ccd_sem, \
                nc.semaphore("ccc_sem") as ccc_sem:
            nc.gpsimd.dma_start(out=cc_in.ap(), in_=cc_stage).then_inc(ccd_sem, 16)
            nc.gpsimd.wait_ge(ccd_sem, 16)
            nc.gpsimd.collective_compute(
                "AllGather", ALU.bypass, replica_groups=[list(range(NCORES))],
                ins=[cc_in.ap()], outs=[cc_out.ap()],
            ).then_inc(ccc_sem, 1)
            nc.gpsimd.wait_ge(ccc_sem, 1)
            nc.gpsimd.dma_start(out=em_row, in_=cc_out.ap()[:, 0:1]
                                .rearrange("r c -> c r")).then_inc(ccd_sem, 16)
            nc.gpsimd.wait_ge(ccd_sem, 32)
        # min of the 8 gathered values -> em scalar
        nc.vector.tensor_reduce(out=em_row[0:1, 0:1], in_=em_row, axis=AX.XYZW,
                                op=ALU.min)
        if DEBUG:
            nc.sync.dma_start(out=dbg["em"].ap(), in_=em_row)
        nc.gpsimd.partition_broadcast(em_col, em_row[0:1, 0:1])
        nc.vector.tensor_scalar_mul(out=nem_col, in0=em_col, scalar1=-1.0)
        nc.vector.tensor_scalar_mul(out=csm, in0=cs_sb, scalar1=em_col[0:64])

        # ================= Phase 5: combine + normalize + w_out ===============
        with (
            tc.tile_pool(name="fsb", bufs=1) as fsb,
            tc.tile_pool(name="fstream", bufs=3) as fstream,
            tc.tile_pool(name="fps", bufs=3, space="PSUM") as fps,
        ):
            s3_4 = fsb.tile([HPC, N], f32)
            nm1 = fsb.tile([1, N], f32)
            nc.sync.dma_start(out=nm1, in_=nmask.ap())
            s3r = fsb.tile([128, HPC * N // 128], f32)
            s3r16 = fsb.tile([128, HPC * N // 128], f16)
            wout_sb = fsb.tile([128, 2, DIM], f16)
            nc.sync.dma_start(out=wout_sb, in_=wout_t)

            # S3 = S2 + em*nm ; reciprocal via partition-reshape
            nc.sync.dma_start(out=s3_4,
                              in_=nmask.ap().partition_broadcast(HPC))
            nc.vector.scalar_tensor_tensor(out=s3_4, in0=s3_4,
                                           scalar=em_col[0:HPC], in1=s2_sb,
                                           op0=ALU.mult, op1=ALU.add)
            nc.sync.dma_start(
                out=scr_c.ap().rearrange("one (h r) -> (one h) r", h=HPC),
                in_=s3_4)
            nc.sync.dma_start(
                out=s3r, in_=scr_c.ap().rearrange("one (po f) -> (one po) f",
                                                  po=128))
            nc.vector.reciprocal(out=s3r, in_=s3r)
            nc.vector.tensor_copy(out=s3r16, in_=s3r)
            nc.sync.dma_start(
                out=scr_d.ap().rearrange("one (po f) -> (one po) f", po=128),
                in_=s3r16)

            for h in range(HPC):
                hc = h // 2
                mv_h = mv_sb[0:64, hc] if h % 2 == 0 else mv_odd[:, hc]
                # T = (Mv * -em) + U2 ; U3 = T + em*colsum
                t_sb = fstream.tile([64, N], f16, tag="t")
                nc.vector.scalar_tensor_tensor(
                    out=t_sb, in0=mv_h, scalar=nem_col[0:64],
                    in1=u2_sb[:, h], op0=ALU.mult, op1=ALU.add)
                nc.vector.tensor_scalar_add(out=t_sb, in0=t_sb,
                                            scalar1=csm[:, h:h + 1])
                # reciprocal row for head h -> broadcast from DRAM
                rb = fstream.tile([64, N], f16, tag="rb")
                nc.sync.dma_start(
                    out=rb, in_=scr_d.ap()[:, h * N:(h + 1) * N]
                    .partition_broadcast(64))
                if h % 2 == 0:
                    tgt = outT_sb[0:64, hc]
                else:
                    tgt = fstream.tile([64, N], f16, tag="odd")
                nc.vector.tensor_mul(out=tgt, in0=t_sb, in1=rb)
                if h % 2 == 1:
                    nc.sync.dma_start(out=outT_sb[64:128, hc], in_=tgt)

            for rc in range(16):
                ps_f = fps.tile([128, 1024], f32, tag="f")
                rsl = slice(rc * 128, (rc + 1) * 128)
                for odc in range(2):
                    osl = slice(odc * 512, (odc + 1) * 512)
                    for pr in range(2):
                        nc.tensor.matmul(ps_f[:, osl], outT_sb[:, pr, rsl],
                                         wout_sb[:, pr, osl],
                                         start=(pr == 0), stop=(pr == 1))
                fout = fstream.tile([128, 1024], f32, tag="fout")
                nc.scalar.copy(out=fout, in_=ps_f)
                nc.sync.dma_start(out=out_t[:, rc], in_=fout)


_NC_CACHE = None


def _get_nc():
    global _NC_CACHE
    if _NC_CACHE is None:
        _NC_CACHE = build_nc()
    return _NC_CACHE


def _prep_inputs(x, mask, gamma, beta, w_qkv, w_out):
    """Build the 8 per-core input maps (all numpy, host-side)."""
    x = np.asarray(x, dtype=np.float32)
    mask = np.asarray(mask)
    gamma = np.asarray(gamma, dtype=np.float32)
    beta = np.asarray(beta, dtype=np.float32)
    w_qkv = np.asarray(w_qkv, dtype=np.float32)
    w_out = np.asarray(w_out, dtype=np.float32)

    in_maps = []
    maskT_f16 = [np.ascontiguousarray(mask[b].T).astype(np.float16)
                 for b in range(B)]
    nmask = [(~mask[b]).sum(axis=1).astype(np.float32)[None, :] for b in range(B)]
    xT = [np.ascontiguousarray(x[b].T).astype(np.float16) for b in range(B)]
    wg = w_qkv * gamma[:, None]          # fold gamma into W rows
    bsum = beta @ w_qkv                  # [3*INNER]
    for c in range(NCORES):
        b = c // GROUPS
        g = c % GROUPS
        cols = []
        for m in range(3):               # q, k, v blocks
            lo = m * INNER + g * HPC * DIM_HEAD
            cols.append(np.arange(lo, lo + HPC * DIM_HEAD))
        cols = np.concatenate(cols)
        Wc = np.ascontiguousarray(wg[:, cols]).astype(np.float16)
        wr1c = np.stack([wg[:, cols].sum(axis=0), bsum[cols]]).astype(np.float16)
        wo = w_out[g * HPC * DIM_HEAD:(g + 1) * HPC * DIM_HEAD, :]
        wo = np.ascontiguousarray(wo.reshape(2, 128, DIM)).astype(np.float16)
        in_maps.append({
            "xT": xT[b],
            "maskT": maskT_f16[b],
            "nmask": nmask[b],
            "wqkv": Wc,
            "wr1": wr1c,
            "wout": wo,
        })
    return in_maps


def kernel(x, mask, gamma, beta, w_qkv, w_out, _trace=False):
    nc = _get_nc()
    in_maps = _prep_inputs(x, mask, gamma, beta, w_qkv, w_out)
    res = run_bass_kernel_spmd(nc, in_maps, core_ids=list(range(NCORES)),
                               trace=_trace)
    kernel.last_result = res
    final = np.zeros((B, N, DIM), dtype=np.float32)
    for c in range(NCORES):
        final[c // GROUPS] += res.results[c]["out_part"]
    return final


# revision 49
# speedup vs baseline: 1.0511x; 1.0511x over previous
"""Fused LayerNorm + 16-head attention (b=2, n=2048, d=1024) on 8 trn2 cores.

Sharding: core c handles batch c//4 and heads [4*(c%4), 4*(c%4)+4).
Each core computes LayerNorm + QKV projection for its batch, attention for
its 4 heads, and a partial w_out product; the host sums the 4 partials per
batch.

Pipeline (all "transposed" layouts, contraction always on the partition dim):
  - host sends x^T [1024, 2048] fp16 and mask^T [2048, 2048] fp16 per batch
  - LN stats (mean/var over dim) via ones-matmuls on the PE
  - LN scale and gamma/beta are folded into the QKV projection:
      qkv^T[c, r] = sum_d W'[d,c] * (x^T[d,r]*rstd[r]) + wsum[c]*(-mu*rstd)[r]
                    + bsum[c]*1
    (W' = gamma*w_qkv columns for this core; wsum/bsum host-precomputed)
  - dots^T[j, i] per head via fp16 matmul (K=64)
  - e1 = exp(SCALE*dots) (ACT, psum->sbuf fp16); e2 = e1*mask^T (DVE fp16 2x)
  - U2[d|1, i] = sum_j [v|1][j,d] * e2[j,i]  (fp16 matmul, K=128 chunks),
    software-pipelined one j-chunk behind the dots matmuls
  - masked-fill-with-global-min handled exactly via
      U3 = U2 + em*(colsum_v - mask^T@v),  S3 = S2 + em*n_masked
    where em = exp(SCALE*min(dots)) = global min of e1 (running TT-min trees
    split across DVE and GPSIMD) + an 8-core AllReduce(min) collective.
  - out^T = U3 * (1/S3) ; partial = out^T.T @ w_out rows  (fp16 matmul)
"""

import numpy as np
import ml_dtypes

import concourse.bass as bass
import concourse.mybir as mybir
import concourse.tile as tile
from concourse import bacc
from concourse import bass_isa
from concourse.bass_utils import run_bass_kernel_spmd
from concourse.masks import make_identity

DIM, HEADS, DIM_HEAD = 1024, 16, 64
B, N = 2, 2048
INNER = HEADS * DIM_HEAD
SCALE = DIM_HEAD ** -0.5
EPS = 1e-5
NCORES = 8
GROUPS = NCORES // B          # head-groups per batch = 4
HPC = HEADS // GROUPS         # heads per core = 4
WCOLS = 3 * HPC * DIM_HEAD    # 768 qkv columns per core
KCH = DIM // 128              # 8 contraction chunks
NJC = N // 128                # 16 j-chunks
NIC = N // 512                # 4 i-chunks of 512
IH = 1024                     # i-half width

f32 = mybir.dt.float32
f16 = mybir.dt.float16
AX = mybir.AxisListType
ALU = mybir.AluOpType
ACTF = mybir.ActivationFunctionType

DEBUG = False


def build_nc():
    nc = bacc.Bacc(None, target_bir_lowering=False)

    xT = nc.dram_tensor("xT", [DIM, N], f16, kind="ExternalInput")
    maskT = nc.dram_tensor("maskT", [N, N], f16, kind="ExternalInput")
    nmask = nc.dram_tensor("nmask", [1, N], f32, kind="ExternalInput")
    wqkv = nc.dram_tensor("wqkv", [DIM, WCOLS], f16, kind="ExternalInput")
    wr1 = nc.dram_tensor("wr1", [2, WCOLS], f16, kind="ExternalInput")
    wout = nc.dram_tensor("wout", [2, 128, DIM], f16, kind="ExternalInput")
    out = nc.dram_tensor("out_part", [N, DIM], f32, kind="ExternalOutput")
    cc_in = nc.dram_tensor("cc_in", [1, 8], f32)
    cc_out = nc.dram_tensor("cc_out", [8, 8], f32, addr_space="Shared")
    scr_a = nc.dram_tensor("scr_a", [1, N], f32)
    scr_b = nc.dram_tensor("scr_b", [1, N], f16)
    scr_c = nc.dram_tensor("scr_c", [1, HPC * N], f32)
    scr_d = nc.dram_tensor("scr_d", [1, HPC * N], f16)
    scr_e = nc.dram_tensor("scr_e", [1, HPC * N], f32)
    scr_f = nc.dram_tensor("scr_f", [1, HPC * N], f16)
    dbg = {}
    if DEBUG:
        dbg["em"] = nc.dram_tensor("dbg_em", [1, 8], f32, kind="ExternalOutput")
        dbg["qk"] = nc.dram_tensor("dbg_qk", [128, 4, N], f16, kind="ExternalOutput")
        dbg["u2"] = nc.dram_tensor("dbg_u2", [64, HPC, 2, IH], f16, kind="ExternalOutput")
        dbg["s2"] = nc.dram_tensor("dbg_s2", [HPC, N], f32, kind="ExternalOutput")
        dbg["mv"] = nc.dram_tensor("dbg_mv", [128, 2, N], f16, kind="ExternalOutput")
        dbg["acc"] = nc.dram_tensor("dbg_acc", [128, IH], f16, kind="ExternalOutput")
        dbg["outT"] = nc.dram_tensor("dbg_outT", [128, 2, N], f16, kind="ExternalOutput")
        dbg["stats"] = nc.dram_tensor("dbg_stats", [2, N], f32, kind="ExternalOutput")
        dbg["rstd16"] = nc.dram_tensor("dbg_rstd16", [1, N], f16, kind="ExternalOutput")

    scr = (scr_a, scr_b, scr_c, scr_d, scr_e, scr_f)
    with tile.TileContext(nc) as tc:
        _build_body(nc, tc, xT, maskT, nmask, wqkv, wr1, wout, out,
                    cc_in, cc_out, scr, dbg)
    nc.compile()
    return nc


def _build_body(nc, tc, xT, maskT, nmask, wqkv, wr1, wout, out, cc_in, cc_out,
                scr, dbg):
    scr_a, scr_b, scr_c, scr_d, scr_e, scr_f = scr
    xT_t = xT.ap().rearrange("(ko p) r -> p ko r", p=128)          # [128, 8, N]
    maskT_t = maskT.ap().rearrange("(jo p) i -> p jo i", p=128)    # [128, 16, N]
    wqkv_t = wqkv.ap().rearrange("(ko p) c -> p ko c", p=128)      # [128, 8, 768]
    wout_t = wout.ap().rearrange("pr p od -> p pr od")             # [128, 2, 1024]
    out_t = out.ap().rearrange("(ro p) od -> p ro od", p=128)      # [128, 16, 1024]

    with tc.tile_pool(name="L1", bufs=1) as L1:
        # ---- whole-kernel tensors ----
        u2_sb = L1.tile([64, HPC, N], f16)             # U2 rows fp16 (16KB/p)
        s2st = L1.tile([65, IH], f32)                  # S-row staging (4KB/p)
        s2_sb = L1.tile([HPC, N], f32)
        mv_sb = L1.tile([128, 2, N], f16)
        outT_sb = L1.tile([128, 2, N], f16)
        identones = L1.tile([128, 132], f16)           # ident | ones col
        sm32 = L1.tile([128, 16], f32)   # 0:em_col 1:nem_col 2-5:cs 6-9:csm
                                         # [0,10]:eps [0,11]:local_min
        smrow = L1.tile([1, 16], f32)    # 0-7: cc_stage, 8-15: em_row

        ident = identones[:, 0:128]
        ones16 = identones[:, 128:129]
        em_col = sm32[:, 0:1]
        nem_col = sm32[:, 1:2]
        cs_sb = sm32[0:64, 2:6]
        csm = sm32[0:64, 6:10]
        eps_t = sm32[0:1, 10:11]
        local_min = sm32[0:1, 11:12]
        cc_stage = smrow[0:1, 0:8]
        em_row = smrow[0:1, 8:16]

        nc.vector.memset(identones, 0.0)
        make_identity(nc, ident, nomemset=True)
        nc.vector.memset(ones16, 1.0)
        nc.vector.memset(eps_t, EPS)

        with tc.tile_pool(name="L2", bufs=1) as L2:
            qk_sb = L2.tile([128, 4, N], f16)
            v_aug = L2.tile([128, NJC, 2, 2, 65], f16)
            v_pair = L2.tile([128, NJC, 256], f16)
            acc_min = L2.tile([128, IH], f16)
            w_sb = L2.tile([128, KCH, WCOLS], f16)
            wr1_sb = L2.tile([2, WCOLS], f16)
            nc.sync.dma_start(out=w_sb, in_=wqkv_t)
            nc.sync.dma_start(out=wr1_sb, in_=wr1.ap())
            nc.vector.memset(v_aug[:, :, :, :, 64:65], 1.0)
            nc.vector.memset(acc_min, 60000.0)
            lnout_cm = tc.tile_pool(name="LNOUT", bufs=1)
            LNOUT = lnout_cm.__enter__()
            rstd16 = LNOUT.tile([1, N], f16)
            r1 = LNOUT.tile([2, N], f16)
            x16 = LNOUT.tile([128, KCH, N], f16)
            # ================= Phase 1: LN stats =================
            with (
                tc.tile_pool(name="ph1ps", bufs=1, space="PSUM") as ph1ps,
                tc.tile_pool(name="ph1sb", bufs=1) as ph1sb,
                tc.tile_pool(name="xstream", bufs=3) as xstream,
            ):
                ps_sx = ph1ps.tile([1, N], f32)
                ps_sxx = ph1ps.tile([1, N], f32)
                for k in range(KCH):
                    nc.sync.dma_start(out=x16[:, k], in_=xT_t[:, k])
                    sq = xstream.tile([128, N], f16, tag="sq")
                    nc.vector.tensor_mul(out=sq, in0=x16[:, k], in1=x16[:, k])
                    for s in range(NIC):
                        sl = slice(s * 512, (s + 1) * 512)
                        nc.tensor.matmul(ps_sx[:, sl], ones16, x16[:, k, sl],
                                         start=(k == 0), stop=(k == KCH - 1))
                    for s in range(NIC):
                        sl = slice(s * 512, (s + 1) * 512)
                        nc.tensor.matmul(ps_sxx[:, sl], ones16, sq[:, sl],
                                         start=(k == 0), stop=(k == KCH - 1))
                # stats on partition 0
                mu = ph1sb.tile([1, N], f32)
                ex2 = ph1sb.tile([1, N], f32)
                mu2 = ph1sb.tile([1, N], f32)
                nc.vector.tensor_scalar_mul(out=mu, in0=ps_sx, scalar1=1.0 / DIM)
                nc.vector.tensor_scalar_mul(out=ex2, in0=ps_sxx, scalar1=1.0 / DIM)
                nc.scalar.activation(out=mu2, in_=mu, func=ACTF.Square)
                nc.vector.tensor_sub(out=ex2, in0=ex2, in1=mu2)     # var
                nc.scalar.activation(out=ex2, in_=ex2, func=ACTF.Sqrt,
                                     bias=eps_t)                    # sqrt(var+eps)
                # fast reciprocal: bounce through DRAM to use 128 partitions
                nc.sync.dma_start(out=scr_a.ap(), in_=ex2)
                rr = ph1sb.tile([128, N // 128], f32)
                nc.sync.dma_start(
                    out=rr, in_=scr_a.ap().rearrange("one (po f) -> (one po) f", po=128))
                nc.vector.reciprocal(out=rr, in_=rr)
                rr16 = ph1sb.tile([128, N // 128], f16)
                nc.vector.tensor_copy(out=rr16, in_=rr)
                nc.sync.dma_start(
                    out=scr_b.ap().rearrange("one (po f) -> (one po) f", po=128), in_=rr16)
                nc.sync.dma_start(out=rstd16, in_=scr_b.ap())
                # r1 row0 = -mu (f16); row1 = sd = 1/rstd (so that
                # rstd*(bsum*sd) = bsum at evac time)
                nc.vector.tensor_scalar_mul(out=mu2, in0=mu, scalar1=-1.0)
                nc.vector.tensor_copy(out=r1[0:1, :], in_=mu2)
                nc.gpsimd.dma_start(out=r1[1:2, :], in_=ex2)
                if DEBUG:
                    nc.sync.dma_start(out=dbg["stats"].ap()[0:1], in_=mu)
                    nc.sync.dma_start(out=dbg["rstd16"].ap(), in_=rstd16)

            # ================= Phase 2 wrapper =================
            if True:
                # ================= Phase 2: xhat + QKV projection ==============
                with tc.tile_pool(name="ph2sb", bufs=1) as ph2sb:
                    vT_sb = ph2sb.tile([128, 2, N], f16)
                    rstd_bc = ph2sb.tile([128, N], f16)
                    nc.gpsimd.partition_broadcast(rstd_bc, rstd16)

                    # proj on x16 directly; LN scale applied at psum evac.
                    # r1 row0 here is -mu (rstd folded at evac): see below.
                    ph2ps_cm = tc.tile_pool(name="ph2ps", bufs=2, space="PSUM")
                    ph2ps = ph2ps_cm.__enter__()
                    for cc in range(6):
                        ps_p = ph2ps.tile([128, N], f32, tag="proj")
                        ccs = slice(cc * 128, (cc + 1) * 128)
                        for k in range(KCH + 1):
                            lhsT = w_sb[:, k, ccs] if k < KCH else wr1_sb[:, ccs]
                            for s in range(NIC):
                                sl = slice(s * 512, (s + 1) * 512)
                                rhs = x16[:, k, sl] if k < KCH else r1[:, sl]
                                nc.tensor.matmul(ps_p[:, sl], lhsT, rhs,
                                                 start=(k == 0), stop=(k == KCH))
                        tgt = qk_sb[:, cc] if cc < 4 else vT_sb[:, cc - 4]
                        nc.vector.tensor_mul(out=tgt, in0=ps_p, in1=rstd_bc)

                    ph2ps_cm.__exit__(None, None, None)
                    # ---- v transposes -> v_aug natural layout
                    with tc.tile_pool(name="tps", bufs=2, space="PSUM") as tps:
                        for jc in range(NJC):
                            ps_t = tps.tile([128, 256], f16, tag="vt")
                            jsl = slice(jc * 128, (jc + 1) * 128)
                            for hc in range(2):
                                nc.tensor.transpose(
                                    ps_t[:, hc * 128:(hc + 1) * 128],
                                    vT_sb[:, hc, jsl], ident)
                            nc.vector.tensor_copy(
                                out=v_aug[:, jc, :, :, 0:64],
                                in_=ps_t.rearrange("p (pr m d) -> p pr m d",
                                                   pr=2, m=2))
                            nc.vector.tensor_copy(out=v_pair[:, jc], in_=ps_t)
                        # per-head column sums of v
                        for h in range(HPC):
                            ps_cs = tps.tile([64, 1], f32, tag="cs")
                            for jc in range(NJC):
                                nc.tensor.matmul(ps_cs,
                                                 v_aug[:, jc, h // 2, h % 2, 0:64],
                                                 ones16, start=(jc == 0),
                                                 stop=(jc == NJC - 1))
                            nc.vector.tensor_copy(out=cs_sb[:, h:h + 1], in_=ps_cs)

                lnout_cm.__exit__(None, None, None)
                # ================= Phase 3: attention (full mask resident) =====
                with (
                    tc.tile_pool(name="dps", bufs=2, space="PSUM") as dps,
                    tc.tile_pool(name="ups", bufs=1, space="PSUM") as ups,
                    tc.tile_pool(name="mvps", bufs=1, space="PSUM") as mvps,
                    tc.tile_pool(name="epool", bufs=4) as epool,
                    tc.tile_pool(name="maskp", bufs=1) as maskp,
                ):
                    mask_sb = maskp.tile([128, NJC, N], f16)
                    for jo in range(NJC):
                        nc.sync.dma_start(out=mask_sb[:, jo], in_=maskT_t[:, jo])
                    for ih in range(2):
                        isl = slice(ih * IH, (ih + 1) * IH)
                        for h in range(HPC):
                            hc, off = h // 2, (h % 2) * 64
                            qT = qk_sb[off:off + 64, hc]       # [64, N]
                            kT = qk_sb[off:off + 64, 2 + hc]   # [64, N]
                            ps_u = ups.tile([128, IH], f32, tag="u")
                            mv_h = (ih == 0 and (h == 1 or h == 3))
                            if mv_h:
                                pr = h // 2
                                ps_mv = mvps.tile([128, IH], f32, tag="mv")
                            pend = []
                            for jc in range(NJC):
                                jsl = slice(jc * 128, (jc + 1) * 128)
                                ps_d = dps.tile([128, IH], f32, tag="d")
                                for s in range(2):
                                    ssl = slice(ih * IH + s * 512,
                                                ih * IH + (s + 1) * 512)
                                    nc.tensor.matmul(
                                        ps_d[:, s * 512:(s + 1) * 512],
                                        kT[:, jsl], qT[:, ssl],
                                        start=True, stop=True)
                                if mv_h:
                                    # fill PE bubble with this i-half's Mv
                                    for s in range(2):
                                        nc.tensor.matmul(
                                            ps_mv[:, s * 512:(s + 1) * 512],
                                            v_pair[:, jc,
                                                   pr * 128:(pr + 1) * 128],
                                            mask_sb[:, jc, ih * IH + s * 512:
                                                    ih * IH + (s + 1) * 512],
                                            start=(jc == 0),
                                            stop=(jc == NJC - 1))
                                # av lags by 2 j-chunks for PE density
                                if len(pend) == 2:
                                    pjc, pe2 = pend.pop(0)
                                    for s in range(2):
                                        nc.tensor.matmul(
                                            ps_u[0:65, s * 512:(s + 1) * 512],
                                            v_aug[:, pjc, hc, h % 2, :],
                                            pe2[:, s * 512:(s + 1) * 512],
                                            start=(pjc == 0), stop=False)
                                e1 = epool.tile([128, IH], f16, tag="e1")
                                nc.scalar.activation(out=e1, in_=ps_d,
                                                     func=ACTF.Exp, scale=SCALE)
                                e2 = epool.tile([128, IH], f16, tag="e2")
                                nc.vector.tensor_mul(out=e2, in0=e1,
                                                     in1=mask_sb[:, jc, isl])
                                nc.vector.tensor_tensor(
                                    out=acc_min, in0=acc_min,
                                    in1=e1, op=ALU.min)
                                pend.append((jc, e2))
                            for pjc, pe2 in pend:
                                for s in range(2):
                                    nc.tensor.matmul(
                                        ps_u[0:65, s * 512:(s + 1) * 512],
                                        v_aug[:, pjc, hc, h % 2, :],
                                        pe2[:, s * 512:(s + 1) * 512],
                                        start=(pjc == 0),
                                        stop=(pjc == NJC - 1))
                            nc.scalar.copy(out=u2_sb[:, h, isl],
                                           in_=ps_u[0:64])
                            nc.scalar.copy(out=s2st[64:65],
                                           in_=ps_u[64:65])
                            nc.sync.dma_start(out=s2_sb[h:h + 1, isl],
                                              in_=s2st[64:65])
                            if mv_h:
                                nc.scalar.copy(out=mv_sb[:, pr, isl],
                                               in_=ps_mv)
                        if ih == 1:
                            # trailing Mv chains: run while the global-min
                            # collective is in flight (no e1 dependency)
                            for pr in range(2):
                                ps_mv = mvps.tile([128, IH], f32, tag="mv")
                                for jc in range(NJC):
                                    for s in range(2):
                                        nc.tensor.matmul(
                                            ps_mv[:, s * 512:(s + 1) * 512],
                                            v_pair[:, jc,
                                                   pr * 128:(pr + 1) * 128],
                                            mask_sb[:, jc, ih * IH + s * 512:
                                                    ih * IH + (s + 1) * 512],
                                            start=(jc == 0),
                                            stop=(jc == NJC - 1))
                                nc.scalar.copy(out=mv_sb[:, pr, isl],
                                               in_=ps_mv)

                if DEBUG:
                    nc.sync.dma_start(out=dbg["qk"].ap(), in_=qk_sb)
                    nc.sync.dma_start(out=dbg["acc"].ap(), in_=acc_min)
                # collapse acc_min tree -> local_min scalar
                pm = L2.tile([128, 1], f32)
                nc.vector.tensor_reduce(out=pm, in_=acc_min, axis=AX.XYZW,
                                        op=ALU.min)
                nc.vector.tensor_scalar_mul(out=pm, in0=pm, scalar1=-1.0)
                pmx = L2.tile([128, 1], f32)
                nc.gpsimd.partition_all_reduce(pmx, pm, channels=128,
                                               reduce_op=bass_isa.ReduceOp.max)
                nc.vector.tensor_scalar_mul(out=local_min, in0=pmx[0:1],
                                            scalar1=-1.0)

        # ================= Phase 4: global-min collective =================
        mv_odd = L1.tile([64, 2, N], f16)
        for pr in range(2):
            nc.sync.dma_start(out=mv_odd[:, pr], in_=mv_sb[64:128, pr])
        nc.vector.memset(cc_stage, 3.0e38)
        nc.vector.tensor_copy(out=cc_stage[0:1, 0:1], in_=local_min)
        with tc.tile_critical(), nc.semaphore("ccd_sem") as ccd_sem, \
                nc.semaphore("ccc_sem") as ccc_sem:
            nc.gpsimd.dma_start(out=cc_in.ap(), in_=cc_stage).then_inc(ccd_sem, 16)
            nc.gpsimd.wait_ge(ccd_sem, 16)
            nc.gpsimd.collective_compute(
                "AllGather", ALU.bypass, replica_groups=[list(range(NCORES))],
                ins=[cc_in.ap()], outs=[cc_out.ap()],
            ).then_inc(ccc_sem, 1)
            nc.gpsimd.wait_ge(ccc_sem, 1)
            nc.gpsimd.dma_start(out=em_row, in_=cc_out.ap()[:, 0:1]
                                .rearrange("r c -> c r")).then_inc(ccd_sem, 16)
            nc.gpsimd.wait_ge(ccd_sem, 32)
        # min of the 8 gathered values -> em scalar
        nc.vector.tensor_reduce(out=em_row[0:1, 0:1], in_=em_row, axis=AX.XYZW,
                                op=ALU.min)
        if DEBUG:
            nc.sync.dma_start(out=dbg["em"].ap(), in_=em_row)
        nc.gpsimd.partition_broadcast(em_col, em_row[0:1, 0:1])
        nc.vector.tensor_scalar_mul(out=nem_col, in0=em_col, scalar1=-1.0)
        nc.vector.tensor_scalar_mul(out=csm, in0=cs_sb, scalar1=em_col[0:64])

        # ================= Phase 5: combine + normalize + w_out ===============
        with (
            tc.tile_pool(name="fsb", bufs=1) as fsb,
            tc.tile_pool(name="fstream", bufs=3) as fstream,
            tc.tile_pool(name="fps", bufs=3, space="PSUM") as fps,
        ):
            s3_4 = fsb.tile([HPC, N], f32)
            nm1 = fsb.tile([1, N], f32)
            nc.sync.dma_start(out=nm1, in_=nmask.ap())
            s3r = fsb.tile([128, HPC * N // 128], f32)
            s3r16 = fsb.tile([128, HPC * N // 128], f16)
            wout_sb = fsb.tile([128, 2, DIM], f16)
            nc.sync.dma_start(out=wout_sb, in_=wout_t)

            # S3 = S2 + em*nm ; reciprocal via partition-reshape
            nc.sync.dma_start(out=s3_4,
                              in_=nmask.ap().partition_broadcast(HPC))
            nc.vector.scalar_tensor_tensor(out=s3_4, in0=s3_4,
                                           scalar=em_col[0:HPC], in1=s2_sb,
                                           op0=ALU.mult, op1=ALU.add)
            nc.sync.dma_start(
                out=scr_c.ap().rearrange("one (h r) -> (one h) r", h=HPC),
                in_=s3_4)
            nc.sync.dma_start(
                out=s3r, in_=scr_c.ap().rearrange("one (po f) -> (one po) f",
                                                  po=128))
            nc.vector.reciprocal(out=s3r, in_=s3r)
            nc.vector.tensor_copy(out=s3r16, in_=s3r)
            nc.sync.dma_start(
                out=scr_d.ap().rearrange("one (po f) -> (one po) f", po=128),
                in_=s3r16)

            for h in range(HPC):
                hc = h // 2
                mv_h = mv_sb[0:64, hc] if h % 2 == 0 else mv_odd[:, hc]
                # T = (Mv * -em) + U2 ; U3 = T + em*colsum
                t_sb = fstream.tile([64, N], f16, tag="t")
                nc.vector.scalar_tensor_tensor(
                    out=t_sb, in0=mv_h, scalar=nem_col[0:64],
                    in1=u2_sb[:, h], op0=ALU.mult, op1=ALU.add)
                nc.vector.tensor_scalar_add(out=t_sb, in0=t_sb,
                                            scalar1=csm[:, h:h + 1])
                # reciprocal row for head h -> broadcast from DRAM
                rb = fstream.tile([64, N], f16, tag="rb")
                nc.sync.dma_start(
                    out=rb, in_=scr_d.ap()[:, h * N:(h + 1) * N]
                    .partition_broadcast(64))
                if h % 2 == 0:
                    tgt = outT_sb[0:64, hc]
                else:
                    tgt = fstream.tile([64, N], f16, tag="odd")
                nc.vector.tensor_mul(out=tgt, in0=t_sb, in1=rb)
                if h % 2 == 1:
                    nc.sync.dma_start(out=outT_sb[64:128, hc], in_=tgt)

            for rc in range(16):
                ps_f = fps.tile([128, 1024], f32, tag="f")
                rsl = slice(rc * 128, (rc + 1) * 128)
                for odc in range(2):
                    osl = slice(odc * 512, (odc + 1) * 512)
                    for pr in range(2):
                        nc.tensor.matmul(ps_f[:, osl], outT_sb[:, pr, rsl],
                                         wout_sb[:, pr, osl],
                                         start=(pr == 0), stop=(pr == 1))
                fout = fstream.tile([128, 1024], f32, tag="fout")
                nc.scalar.copy(out=fout, in_=ps_f)
                nc.sync.dma_start(out=out_t[:, rc], in_=fout)


_NC_CACHE = None


def _get_nc():
    global _NC_CACHE
    if _NC_CACHE is None:
        _NC_CACHE = build_nc()
    return _NC_CACHE


def _prep_inputs(x, mask, gamma, beta, w_qkv, w_out):
    """Build the 8 per-core input maps (all numpy, host-side)."""
    x = np.asarray(x, dtype=np.float32)
    mask = np.asarray(mask)
    gamma = np.asarray(gamma, dtype=np.float32)
    beta = np.asarray(beta, dtype=np.float32)
    w_qkv = np.asarray(w_qkv, dtype=np.float32)
    w_out = np.asarray(w_out, dtype=np.float32)

    in_maps = []
    maskT_f16 = [np.ascontiguousarray(mask[b].T).astype(np.float16)
                 for b in range(B)]
    nmask = [(~mask[b]).sum(axis=1).astype(np.float32)[None, :] for b in range(B)]
    xT = [np.ascontiguousarray(x[b].T).astype(np.float16) for b in range(B)]
    wg = w_qkv * gamma[:, None]          # fold gamma into W rows
    bsum = beta @ w_qkv                  # [3*INNER]
    for c in range(NCORES):
        b = c // GROUPS
        g = c % GROUPS
        cols = []
        for m in range(3):               # q, k, v blocks
            lo = m * INNER + g * HPC * DIM_HEAD
            cols.append(np.arange(lo, lo + HPC * DIM_HEAD))
        cols = np.concatenate(cols)
        Wc = np.ascontiguousarray(wg[:, cols]).astype(np.float16)
        wr1c = np.stack([wg[:, cols].sum(axis=0), bsum[cols]]).astype(np.float16)
        wo = w_out[g * HPC * DIM_HEAD:(g + 1) * HPC * DIM_HEAD, :]
        wo = np.ascontiguousarray(wo.reshape(2, 128, DIM)).astype(np.float16)
        in_maps.append({
            "xT": xT[b],
            "maskT": maskT_f16[b],
            "nmask": nmask[b],
            "wqkv": Wc,
            "wr1": wr1c,
            "wout": wo,
        })
    return in_maps


def kernel(x, mask, gamma, beta, w_qkv, w_out, _trace=False):
    nc = _get_nc()
    in_maps = _prep_inputs(x, mask, gamma, beta, w_qkv, w_out)
    res = run_bass_kernel_spmd(nc, in_maps, core_ids=list(range(NCORES)),
                               trace=_trace)
    kernel.last_result = res
    final = np.zeros((B, N, DIM), dtype=np.float32)
    for c in range(NCORES):
        final[c // GROUPS] += res.results[c]["out_part"]
    return final
